# revision 7
# baseline (speedup 1.0000x reference)
"""Masked multi-head attention on 8 Trainium2 NeuronCores — v2.

Problem (hardcoded): x[4,2048,512] f32, mask[1,4,2048,2048] bool,
Wq/Wk/Wv[512,512] f32.  out = softmax(mask? -inf : (xWq.T)(xWk.T).T/sqrt(128)) @ (xWv.T)
per head (8 heads of dim 64), merged back to [4,2048,512] f32.

Sharding: core c handles batch b=c//2 and head-quad hg=c%2 (heads hg*4..hg*4+3).

v2 changes vs v1 (v1 measured 200µs in the same interleaved HW session where
this version measured 126-156µs, ~25-37% faster; CoreSim timeline model:
157µs/body marginal vs 181µs/body for v1; ACT-exp is the bottleneck engine
at ~134µs busy):
- Epilogue: PE transposes replaced by the DMA XBAR hw transpose (bf16,
  V-width padded 65->80 for the 16-row xbar tile constraint). Per (m, qb):
  one psum->sbuf bf16 copy, one hw-transpose DMA, one batched reciprocal
  [128,8], one broadcast normalize, two output DMAs. Output DRAM tensor is
  bf16, assembled to f32 on host (rel err 4.8e-3, was 4.0e-3).
- Mask multiply: single broadcast tensor_mul over both heads per k-tile.
- All four mask q-blocks prefetched up front on the SP queue (maskp bufs=4):
  a 2MB nm transfer mid-run stalls the epilogue transpose DMA and with it
  the next (m, qb)'s PV matmuls (7µs PE bubbles at block boundaries).
- Inputs land via 5 big DMAs (wq, wk, 2 x-halves, wv) in need-order; HWDGE
  issue costs ~625ns per DMA so fewer+bigger wins.
- 30 junk matmuls warm the PE HAM clock during the initial input DMA wait
  (first body only; at body boundaries the PE is already saturated).
- AV software-pipeline depth 5 (was 2): relieves early-phase PE
  oversubscription so the bottleneck ACT engine starves less.
- x/w/qk pools double-buffered: body i+1's input DMAs and projections
  overlap body i's attention tail. HW-measured (interleaved 3-arm session):
  154.9us (warmup every body, single-buffered) -> 153.2 (body-0 warmup)
  -> 137.9 (plus double-buffered pools).
"""

from collections import deque

import numpy as np
import ml_dtypes

import concourse.bass as bass
import concourse.mybir as mybir
import concourse.tile as tile
from concourse import bacc
from concourse.bass_utils import run_bass_kernel_spmd

BF16 = mybir.dt.bfloat16
F32 = mybir.dt.float32
NPBF16 = ml_dtypes.bfloat16

B, N, C = 4, 2048, 512
H, D = 8, 64
TEMP = float((2.0 * D) ** 0.5)  # sqrt(128)
P = 128
NCORES = 8
HPC = H // 2          # 4 heads per core
DQ = HPC * D          # 256 projection cols per core
KT = N // P           # 16 k tiles
QB = N // 512         # 4 q blocks
VW = 80               # V width incl. ones column, padded to 16-multiple
AV_DEPTH = 5          # software-pipeline depth for PV matmul emission
EPI_GAP = 1           # extra slots between epilogue copy and transpose


def _build_program(repeat=1, ablate=()):
    nc = bacc.Bacc(
        "TRN2",
        target_bir_lowering=False,
        debug=False,
        enable_asserts=False,
        num_devices=NCORES,
    )

    xT = nc.dram_tensor("xT", [C, N], BF16, kind="ExternalInput").ap()
    wqT = nc.dram_tensor("wqT", [C, DQ], BF16, kind="ExternalInput").ap()
    wkT = nc.dram_tensor("wkT", [C, DQ], BF16, kind="ExternalInput").ap()
    wvT = nc.dram_tensor("wvT", [C, DQ], BF16, kind="ExternalInput").ap()
    nmT = nc.dram_tensor("nmT", [N, N], BF16, kind="ExternalInput").ap()
    o = nc.dram_tensor("o", [N, DQ], BF16, kind="ExternalOutput").ap()

    nm_view = nmT.rearrange("(t p) q -> p t q", p=P)  # [128, 16, 2048]

    with tile.TileContext(nc) as tc:
        dbuf = 1 if "nodbuf" in ablate else 2
        with (
            tc.tile_pool(name="xp", bufs=dbuf) as xp,
            tc.tile_pool(name="wp", bufs=dbuf) as wp,
            tc.tile_pool(name="qkvp", bufs=dbuf) as qkvp,
            tc.tile_pool(name="maskp", bufs=4) as maskp,
            tc.tile_pool(name="workp", bufs=8) as workp,
            tc.tile_pool(name="outp", bufs=3) as outp,
            tc.tile_pool(name="psp", bufs=2 if "ps22" in ablate else 3,
                         space="PSUM") as psp,
            tc.tile_pool(name="psot", bufs=2 if "ps22" in ablate else 1,
                         space="PSUM") as psot,
        ):
            for body in range(repeat):
                _emit_body(nc, tc, xT, wqT, wkT, wvT, nm_view, o,
                           xp, wp, qkvp, maskp, workp, outp, psp, psot,
                           ablate=ablate, body=body)

    nc.compile()
    return nc


def _emit_body(nc, tc, xT, wqT, wkT, wvT, nm_view, o,
               xp, wp, qkvp, maskp, workp, outp, psp, psot, ablate=(), body=0):
    # ---- load inputs: few big DMAs (HWDGE issue is ~625ns per DMA), in
    # need order: q/k weights, x, v weights, then the 4 mask q-blocks ----
    wtiles = {w: wp.tile([P, 4, DQ], BF16, name=f"w{w}", tag=f"w{w}")
              for w in ("q", "k", "v")}
    ws = {w: [wtiles[w][:, c, :] for c in range(4)] for w in ("q", "k", "v")}
    xhalf = [xp.tile([P, 2, N], BF16, name=f"xh{i}", tag=f"xh{i}") for i in range(2)]
    xt = [xhalf[c // 2][:, c % 2, :] for c in range(4)]
    for w, dram in (("q", wqT), ("k", wkT)):
        nc.sync.dma_start(out=wtiles[w], in_=dram.rearrange("(c p) d -> p c d", p=P))
    for i in range(2):
        nc.sync.dma_start(
            out=xhalf[i],
            in_=xT[i * 2 * P:(i + 1) * 2 * P, :].rearrange("(c p) n -> p c n", p=P))
    nc.sync.dma_start(out=wtiles["v"], in_=wvT.rearrange("(c p) d -> p c d", p=P))

    if body == 0:
        # Keep the PE busy during the initial input DMA so the HAM clock
        # ramps before the first projection matmuls (body 0 only: in a
        # repeated program the PE is already saturated at body boundaries).
        junk = wp.tile([P, 256], BF16, name="junk", tag="junk")
        nc.gpsimd.memset(junk, 0.0)
        jps = psp.tile([16, 256], F32, name="jps", tag="st")
        for _ in range(30):
            nc.tensor.matmul(jps, lhsT=junk[:, 0:16], rhs=junk, start=True,
                             stop=True)
    # prefetch all mask q-blocks up front: keeps the DMA engines uncontended
    # during the attention loop (a 2MB nm transfer mid-run stalls the
    # epilogue transpose DMA and with it the next (m, qb)'s PV matmuls)
    nm_tiles = []
    for qb in range(QB):
        nm = maskp.tile([P, KT, 512], BF16, name="nm", tag="nm")
        nc.sync.dma_start(out=nm, in_=nm_view[:, :, qb * 512:(qb + 1) * 512])
        nm_tiles.append(nm)

    # ---- projections ----
    # QT/KT in [d', n] layout: partition tile m holds heads (2m, 2m+1).
    qt_sb = [qkvp.tile([P, N], BF16, name=f"qt_sb{m}", tag=f"qt{m}") for m in range(2)]
    kt_sb = [qkvp.tile([P, N], BF16, name=f"kt_sb{m}", tag=f"kt{m}") for m in range(2)]

    def qk_group(wname, m, nb):
        t = (qt_sb if wname == "q" else kt_sb)[m]
        ps = psp.tile([P, 512], F32, name="proj_ps", tag="st")
        for c in range(4):
            nc.tensor.matmul(
                ps,
                lhsT=ws[wname][c][:, m * P:(m + 1) * P],
                rhs=xt[c][:, nb * 512:(nb + 1) * 512],
                start=(c == 0),
                stop=(c == 3),
            )
        nc.vector.tensor_copy(t[:, nb * 512:(nb + 1) * 512], ps)

    # V in [k, d'] layout with a ones column per head: [128, kt*(4*80)]
    vext = qkvp.tile([P, KT * HPC * VW], BF16)
    nc.gpsimd.memset(vext, 1.0)

    def v_group(kti):
        ps = psp.tile([P, DQ], F32, name="v_ps", tag="st")
        for c in range(4):
            nc.tensor.matmul(
                ps,
                lhsT=xt[c][:, kti * P:(kti + 1) * P],
                rhs=ws["v"][c],
                start=(c == 0),
                stop=(c == 3),
            )
        dst_view = vext[:, kti * HPC * VW:(kti + 1) * HPC * VW].rearrange(
            "p (h e) -> p h e", h=HPC
        )[:, :, 0:D]
        src_view = ps.rearrange("p (h e) -> p h e", h=HPC)
        nc.vector.tensor_copy(dst_view, src_view)

    # minimal prelude: only what the first (qb0, m0) scores need right away
    qk_group("q", 0, 0)
    qk_group("k", 0, 0)
    prelude = deque()
    for spec in [("k", 0, 1), ("k", 0, 2), ("k", 0, 3),
                 ("q", 1, 0), ("k", 1, 0), ("k", 1, 1), ("k", 1, 2), ("k", 1, 3),
                 ("q", 1, 1), ("q", 1, 2), ("q", 1, 3),
                 ("q", 0, 1), ("q", 0, 2), ("q", 0, 3)]:
        prelude.append(lambda spec=spec: qk_group(*spec))
    vqueue = deque(lambda kti=kti: v_group(kti) for kti in range(KT))

    # ---- attention (software-pipelined emission) ----
    av_queue = deque()   # deferred PV-matmul emissions
    epi_stages = deque() # deferred epilogue stages of the previous (m, qb)

    def emit_slot():
        if vqueue:
            vqueue.popleft()()
        if prelude:
            prelude.popleft()()
        if len(av_queue) > AV_DEPTH:
            av_queue.popleft()()
        if epi_stages:
            epi_stages.popleft()()

    def make_epilogue(ot, m, qb):
        ctx = {}

        def copy_stage():
            ots = outp.tile([VW, 1024], BF16, name="ots", tag="ots")
            nc.vector.tensor_copy(ots, ot)
            ctx["ots"] = ots
        yield copy_stage

        def transpose_stage():
            otT = outp.tile([P, 8, VW], BF16, name="otT", tag="otT")
            nc.sync.dma_start_transpose(otT, ctx["ots"])
            ctx["otT"] = otT
        yield transpose_stage

        def recip_stage():
            rec = outp.tile([P, 8], F32, name="rec", tag="rec")
            nc.vector.reciprocal(rec, ctx["otT"][:, :, D:D + 1])
            ctx["rec"] = rec
        yield recip_stage

        def norm_stage():
            ob = outp.tile([P, 8, D], BF16, name="ob", tag="ob")
            nc.vector.tensor_mul(
                ob, ctx["otT"][:, :, 0:D],
                ctx["rec"][:, :, None].broadcast_to([P, 8, D]))
            ctx["ob"] = ob
        yield norm_stage

        for hl in range(2):
            def out_stage(hl=hl):
                o_view = o[qb * 512:(qb + 1) * 512,
                           m * P + hl * D:m * P + (hl + 1) * D].rearrange(
                    "(sl p) d -> p sl d", p=P)
                nc.sync.dma_start(out=o_view, in_=ctx["ob"][:, hl * 4:(hl + 1) * 4, :])
            yield out_stage

    for qb in range(QB):
        nm = nm_tiles[qb]
        for m in range(2):
            ot = psot.tile([VW, 1024], F32, name="ot", tag="ot")
            for kti in range(KT):
                st = psp.tile([P, 1024], F32, name="st", tag="st")
                for hl in range(2):
                    nc.tensor.matmul(
                        st[:, hl * 512:(hl + 1) * 512],
                        lhsT=kt_sb[m][hl * D:(hl + 1) * D, kti * P:(kti + 1) * P],
                        rhs=qt_sb[m][hl * D:(hl + 1) * D, qb * 512:(qb + 1) * 512],
                        start=True,
                        stop=True,
                    )
                ex = workp.tile([P, 1024], BF16, name="ex", tag="ex")
                nc.scalar.activation(
                    ex, st, mybir.ActivationFunctionType.Exp, scale=1.0 / TEMP
                )
                exv = ex.rearrange("p (t q) -> p t q", t=2)
                nmb = nm[:, kti, None, :].broadcast_to([P, 2, 512])
                nc.vector.tensor_mul(exv, exv, nmb)

                def av_stage(ot=ot, ex=ex, kti=kti, m=m, qb=qb):
                    for hl in range(2):
                        h = 2 * m + hl
                        nc.tensor.matmul(
                            ot[:, hl * 512:(hl + 1) * 512],
                            lhsT=vext[:, (kti * HPC + h) * VW:(kti * HPC + h + 1) * VW],
                            rhs=ex[:, hl * 512:(hl + 1) * 512],
                            start=(kti == 0),
                            stop=(kti == KT - 1),
                        )
                    if kti == KT - 1:
                        epi_stages.extend(make_epilogue(ot, m, qb))
                av_queue.append(av_stage)
                emit_slot()

    # drain pipeline
    while av_queue:
        av_queue.popleft()()
    while epi_stages:
        epi_stages.popleft()()


_NC_CACHE = {}


def _get_program(repeat=1, ablate=()):
    key = (repeat, tuple(ablate))
    if key not in _NC_CACHE:
        _NC_CACHE[key] = _build_program(repeat, ablate=tuple(ablate))
    return _NC_CACHE[key]


def _make_in_maps(x, mask, Wq, Wk, Wv):
    in_maps = []
    for core in range(NCORES):
        b, hg = core // 2, core % 2
        hsl = slice(hg * DQ, (hg + 1) * DQ)
        in_maps.append({
            "xT": np.ascontiguousarray(x[b].T).astype(NPBF16),
            "wqT": np.ascontiguousarray(Wq[hsl, :].T).astype(NPBF16),
            "wkT": np.ascontiguousarray(Wk[hsl, :].T).astype(NPBF16),
            "wvT": np.ascontiguousarray(Wv[hsl, :].T).astype(NPBF16),
            "nmT": np.ascontiguousarray((~mask[0, b]).T).astype(NPBF16),
        })
    return in_maps


def _assemble(results):
    out = np.empty((B, N, C), dtype=np.float32)
    for core in range(NCORES):
        b, hg = core // 2, core % 2
        out[b, :, hg * DQ:(hg + 1) * DQ] = np.asarray(results[core]["o"]).astype(
            np.float32)
    return out


def run(x, mask, Wq, Wk, Wv, repeat=1, **spmd_kwargs):
    nc = _get_program(repeat)
    in_maps = _make_in_maps(
        np.asarray(x), np.asarray(mask), np.asarray(Wq), np.asarray(Wk), np.asarray(Wv)
    )
    res = run_bass_kernel_spmd(nc, in_maps, list(range(NCORES)), **spmd_kwargs)
    return _assemble(res.results), res


def kernel(x, mask, Wq, Wk, Wv):
    out, _ = run(x, mask, Wq, Wk, Wv)
    return out
